# revision 1
# baseline (speedup 1.0000x reference)
"""MoE layer (top-2 routing, 8 experts) on 8 Trainium2 NeuronCores.

Expert-parallel (one expert per core). Host computes fp32 router logits
only to decide the token->expert all-to-all (the shard decision); all
model math runs on device: per-group bf16 router -> combine weight
w = sigmoid(2*l_e - m1 - m2), bf16 FFN with fp32 PSUM accumulation,
w * (y + b2) applied on device. Output is written in y^T layout
[ND,128,C] (avoids 128x128 PE transposes entirely); the host
untransposes during the return all-to-all scatter-add.

The combine weight w (per-token, [t-partition] layout from the router)
is rebroadcast to a [128, t] tile via one small PE transpose plus
rank-1 ones-matmuls, since the DVE cannot broadcast along partitions.

Weights for the local expert (8 MB W1 + 8 MB W2 bf16) stay resident in
SBUF; tokens stream in groups of 512.
"""

import sys, os

for _p in ("/root/.axon_site", "/root/.axon_site/_ro/trn_rl_repo",
           "/root/.axon_site/_ro/pypackages", "/opt/trn_rl_repo"):
    if os.path.isdir(_p) and _p not in sys.path:
        sys.path.append(_p)

import numpy as np
import ml_dtypes

BF16 = ml_dtypes.bfloat16

T, DIM, E, K, H = 8192, 1024, 8, 2, 4096
N_CORES = 8
ND = DIM // 128             # 8 d-chunks
NH = H // 128               # 32 h-chunks

_compiled = {}


def _build(C):
    from concourse import bass, bacc, tile, mybir
    from concourse.masks import make_identity

    dt = mybir.dt
    NCb = C // 128
    groups = []
    s = 0
    while s < C:
        g = min(512, C - s)
        groups.append((s, g))
        s += g
    NG = len(groups)

    nc = bacc.Bacc("TRN2", target_bir_lowering=False, debug=False,
                   num_devices=N_CORES)

    xbt = nc.dram_tensor("xbt", [NCb, ND, 128, 128], dt.bfloat16, kind="ExternalInput").ap()
    wr = nc.dram_tensor("wr", [DIM, E], dt.bfloat16, kind="ExternalInput").ap()
    w1d = nc.dram_tensor("w1d", [NH, 128, ND, 128], dt.bfloat16, kind="ExternalInput").ap()
    w2d = nc.dram_tensor("w2d", [ND, 128, NH, 128], dt.bfloat16, kind="ExternalInput").ap()
    b1d = nc.dram_tensor("b1d", [128, NH], dt.float32, kind="ExternalInput").ap()
    b2d = nc.dram_tensor("b2d", [128, ND], dt.float32, kind="ExternalInput").ap()
    outT = nc.dram_tensor("outT", [ND, 128, C], dt.float32, kind="ExternalOutput").ap()

    with tile.TileContext(nc) as tc:
        with tc.tile_pool(name="const", bufs=1) as const, \
             tc.tile_pool(name="res", bufs=1) as res, \
             tc.tile_pool(name="xgp", bufs=2) as xgp, \
             tc.tile_pool(name="otp", bufs=3) as otp, \
             tc.tile_pool(name="wbp", bufs=2) as wbp, \
             tc.tile_pool(name="vec", bufs=2) as vec, \
             tc.tile_pool(name="pmm", bufs=4, space="PSUM") as pmm, \
             tc.tile_pool(name="ptr", bufs=2, space="PSUM") as ptr, \
             tc.tile_pool(name="pwb", bufs=1, space="PSUM") as pwb:

            ident = const.tile([128, 128], dt.float32)
            make_identity(nc, ident[:])
            onesb = const.tile([1, 128], dt.float32)
            nc.vector.memset(onesb[:], 1.0)
            wrb = const.tile([128, ND, E], dt.bfloat16)
            b1sb = const.tile([128, NH], dt.float32)
            b2sb = const.tile([128, ND], dt.float32)

            w1sb = res.tile([128, NH, ND, 128], dt.bfloat16)   # 64KB/part
            w2sb = res.tile([128, ND, NH, 128], dt.bfloat16)   # 64KB/part
            hT = res.tile([128, NH, 512], dt.bfloat16)         # 32KB/part
            lg = res.tile([128, NCb, E], dt.float32)
            mx = res.tile([128, NCb, 8], dt.float32)
            wcol = res.tile([128, NCb], dt.float32)
            wts = res.tile([4, 128], dt.float32)
            wrow = res.tile([1, 4, 128], dt.float32)

            xg_tiles = {}

            def load_xg(gi):
                # one descriptor per group: [ntt,ND,128,128] -> [128,ND,ntt,128]
                gs, gn = groups[gi]
                ntt = gn // 128
                tb0 = gs // 128
                xg = xgp.tile([128, ND, 4, 128], dt.bfloat16, tag="xg")
                for tt in range(ntt):
                    nc.sync.dma_start(xg[:, :, tt, :],
                                      xbt[tb0 + tt].transpose([1, 0, 2]))
                xg_tiles[gi] = (xg, gn)

            # group-0 x first (router + l1 both need it), then router weights,
            # then per-chunk W1/W2 so delivery granularity matches consumption
            load_xg(0)
            for dc in range(ND):
                nc.sync.dma_start(wrb[:, dc, :], wr[dc * 128:(dc + 1) * 128, :])
            nc.sync.dma_start(b1sb[:], b1d[:])
            nc.sync.dma_start(b2sb[:], b2d[:])
            for hc in range(NH):
                nc.sync.dma_start(w1sb[:, hc], w1d[hc])
            for dc in range(ND):
                nc.sync.dma_start(w2sb[:, dc], w2d[dc])

            # ---- router: w = sigmoid(2*l_e - m1 - m2), into wcol [t-part] ----
            def emit_router(gi):
                xg, gn = xg_tiles[gi]
                gs = groups[gi][0]
                ntt = gn // 128
                for tt in range(ntt):
                    tb = gs // 128 + tt
                    ps = ptr.tile([128, E], dt.float32, name=f"psr_{tb}", tag="psr")
                    for dc in range(ND):
                        nc.tensor.matmul(ps[:], lhsT=xg[:, dc, tt, :],
                                         rhs=wrb[:, dc, :],
                                         start=(dc == 0), stop=(dc == ND - 1))
                    nc.scalar.copy(lg[:, tb, :], ps[:])
                    nc.vector.max(mx[:, tb, :], lg[:, tb, :])
                tbs = gs // 128
                m_ = vec.tile([128, 4], dt.float32, tag="msum")
                t_ = vec.tile([128, 4], dt.float32, tag="targ")
                nc.vector.tensor_tensor(m_[:, :ntt], mx[:, tbs:tbs + ntt, 0],
                                        mx[:, tbs:tbs + ntt, 1], mybir.AluOpType.add)
                nc.vector.scalar_tensor_tensor(t_[:, :ntt], lg[:, tbs:tbs + ntt, 0],
                                               2.0, m_[:, :ntt],
                                               op0=mybir.AluOpType.mult,
                                               op1=mybir.AluOpType.subtract)
                nc.scalar.activation(wcol[:, tbs:tbs + ntt], t_[:, :ntt],
                                     bass.mybir.ActivationFunctionType.Sigmoid)

            # broadcast w along partitions: PE transpose wcol slice -> rows,
            # flatten to one partition, then rank-1 ones-matmuls -> [128, gn]
            def emit_wT(gi):
                gs, gn = groups[gi]
                ntt = gn // 128
                tbs = gs // 128
                wtp = pwb.tile([4, 128], dt.float32, tag="wtp")
                nc.tensor.transpose(wtp[:ntt, :], wcol[:, tbs:tbs + ntt], ident[:])
                nc.scalar.copy(wts[:ntt, :], wtp[:ntt, :])
                nc.sync.dma_start(wrow[:, :ntt, :], wts[:ntt, :])

            def emit_wbc(gi):
                gs, gn = groups[gi]
                ntt = gn // 128
                wbcp = pwb.tile([128, 512], dt.float32, tag="wbcp")
                for tt in range(ntt):
                    nc.tensor.matmul(wbcp[:, tt * 128:(tt + 1) * 128], lhsT=onesb[:],
                                     rhs=wrow[:, tt, :], start=True, stop=True)
                wbcs = wbp.tile([128, 512], dt.float32, tag="wbcs")
                nc.scalar.copy(wbcs[:, :gn], wbcp[:, :gn])
                return wbcs

            wbcs_t = {}

            def emit_l1(gi):
                xg, gn = xg_tiles[gi]
                ntt = gn // 128
                for hc in range(NH):
                    ps = pmm.tile([128, 512], dt.float32, name=f"ps1_{gi}_{hc}", tag="ps")
                    for dc in range(ND):
                        nc.tensor.matmul(ps[:, :gn], lhsT=w1sb[:, hc, dc, :],
                                         rhs=xg[:, dc, :ntt, :],
                                         start=(dc == 0), stop=(dc == ND - 1))
                    nc.scalar.activation(hT[:, hc, :gn], ps[:, :gn],
                                         bass.mybir.ActivationFunctionType.Gelu,
                                         bias=b1sb[:, hc:hc + 1])
                    if hc == 3:
                        # w-broadcast matmuls injected here: emit_wT(gi) ran
                        # before this group started, so its scalar copy + row
                        # DMA are long done and the PE never waits on them
                        wbcs_t[gi] = emit_wbc(gi)

            def emit_l2(gi):
                gs, gn = groups[gi]
                wbcs = wbcs_t[gi]
                for dc in range(ND):
                    ps = pmm.tile([128, 512], dt.float32, name=f"ps2_{gi}_{dc}", tag="ps")
                    for hc in range(NH):
                        nc.tensor.matmul(ps[:, :gn], lhsT=w2sb[:, dc, hc, :],
                                         rhs=hT[:, hc, :gn],
                                         start=(hc == 0), stop=(hc == NH - 1))
                    osbT = otp.tile([128, 512], dt.float32, tag="osbT")
                    nc.vector.scalar_tensor_tensor(osbT[:, :gn], ps[:, :gn],
                                                   b2sb[:, dc:dc + 1], wbcs[:, :gn],
                                                   op0=mybir.AluOpType.add,
                                                   op1=mybir.AluOpType.mult)
                    nc.sync.dma_start(outT[dc, :, gs:gs + gn], osbT[:, :gn])

            emit_router(0)
            emit_wT(0)
            for gi in range(NG):
                if gi + 1 < NG:
                    load_xg(gi + 1)
                emit_l1(gi)
                if gi + 1 < NG:
                    emit_router(gi + 1)
                emit_l2(gi)
                if gi + 1 < NG:
                    emit_wT(gi + 1)

    nc.compile()
    return nc


def _route(x_flat, Wr):
    logits = x_flat @ Wr                                  # [T, E] fp32
    order = np.argsort(-logits, axis=1)
    top2 = order[:, :K]
    idxs, counts = [], []
    for e in range(E):
        idx = np.nonzero((top2[:, 0] == e) | (top2[:, 1] == e))[0]
        idxs.append(idx)
        counts.append(len(idx))
    C = max(128, -(-max(counts) // 128) * 128)
    return idxs, counts, C


def _prep_inputs(x, Wr, W1, b1, W2, b2, idxs, C):
    x_flat = np.ascontiguousarray(np.asarray(x, np.float32)).reshape(T, DIM)
    Wr = np.ascontiguousarray(np.asarray(Wr, np.float32))
    W1 = np.asarray(W1, np.float32)
    b1 = np.asarray(b1, np.float32)
    W2 = np.asarray(W2, np.float32)
    b2 = np.asarray(b2, np.float32)
    NCb = C // 128

    in_maps = []
    for e in range(E):
        xg = np.zeros((C, DIM), np.float32)
        xg[:len(idxs[e])] = x_flat[idxs[e]]
        xt = np.ascontiguousarray(
            xg.reshape(NCb, 128, ND, 128).transpose(0, 2, 3, 1)).astype(BF16)
        perm = [e] + [j for j in range(E) if j != e]
        in_maps.append({
            "xbt": xt,
            "wr": np.ascontiguousarray(Wr[:, perm]).astype(BF16),
            "w1d": np.ascontiguousarray(
                W1[e].astype(BF16).reshape(ND, 128, NH, 128).transpose(2, 1, 0, 3)),
            "w2d": np.ascontiguousarray(
                W2[e].astype(BF16).reshape(NH, 128, ND, 128).transpose(2, 1, 0, 3)),
            "b1d": np.ascontiguousarray(b1[e].reshape(NH, 128).T),
            "b2d": np.ascontiguousarray(b2[e].reshape(ND, 128).T),
        })
    return in_maps


def kernel(x, Wr, W1, b1, W2, b2, _profile=None):
    global _compiled
    from concourse.bass_utils import run_bass_kernel_spmd

    x_flat = np.ascontiguousarray(np.asarray(x, np.float32)).reshape(T, DIM)
    idxs, counts, C = _route(x_flat, np.asarray(Wr, np.float32))
    if C not in _compiled:
        _compiled[C] = _build(C)
    nc = _compiled[C]
    in_maps = _prep_inputs(x, Wr, W1, b1, W2, b2, idxs, C)
    kwargs = {}
    if _profile:
        kwargs = dict(trace=True, tmpdir=_profile)
    res = run_bass_kernel_spmd(nc, in_maps, core_ids=list(range(N_CORES)), **kwargs)
    full = np.zeros((T, DIM), np.float32)
    for e in range(E):
        yT = np.asarray(res.results[e]["outT"])           # [ND, 128, C]
        y = yT.transpose(2, 0, 1).reshape(C, DIM)
        full[idxs[e]] += y[:counts[e]].astype(np.float32)
    full = full.reshape(4, 2048, DIM)
    if _profile:
        return full, res
    return full



# revision 2
# speedup vs baseline: 1.0713x; 1.0713x over previous
"""MoE layer (top-2 routing, 8 experts) on 8 Trainium2 NeuronCores.

Hidden-dim sharding for perfect load balance: every core holds an H/8
slice (512 wide) of ALL 8 experts' W1/W2 (16.8 MB bf16, SBUF-resident)
and processes ALL 16384 (token, expert) pairs for its slice — so the
per-core matmul column count is exactly 16384*64 regardless of routing
imbalance, and the SPMD program is bit-identical across cores (only
the weight-slice contents differ per core).

The host computes router logits (it must anyway, to decide the
dispatch), gathers tokens by expert into one x^T stream, and applies
the top-2 combine weights + cross-core reduction during the return
scatter-add.  The device runs only the FFN: per <=512-token sub-tile,
GEMM1 (4 h-chunk chains x 8 d-matmuls) -> exact-GELU(+b1) on the
scalar engine -> GEMM2 (8 d-chunk chains x 4 h-matmuls) -> bf16 cast
on the vector engine -> y^T DMA out.  l1(k+1) is emitted before l2(k)
(one-deep software pipeline) so the PE never waits on the gelu drain.
"""

import sys, os

for _p in ("/root/.axon_site", "/root/.axon_site/_ro/trn_rl_repo",
           "/root/.axon_site/_ro/pypackages", "/opt/trn_rl_repo"):
    if os.path.isdir(_p) and _p not in sys.path:
        sys.path.append(_p)

import numpy as np
import ml_dtypes

BF16 = ml_dtypes.bfloat16

T, DIM, E, K, H = 8192, 1024, 8, 2, 4096
N_CORES = 8
ND = DIM // 128             # 8 d-chunks
HS = H // N_CORES           # 512 hidden dims per core
NHL = HS // 128             # 4 h-chunks per core
LT = 512                    # sub-tile token length

_compiled = {}


def _build(G):
    from concourse import bass, bacc, tile, mybir

    dt = mybir.dt
    R = sum(G)

    # sub-tiles: (expert, global row start, length)
    sts = []
    off = 0
    for e in range(E):
        s = 0
        while s < G[e]:
            L = min(LT, G[e] - s)
            sts.append((e, off + s, L))
            s += L
        off += G[e]
    n = len(sts)

    nc = bacc.Bacc("TRN2", target_bir_lowering=False, debug=False,
                   num_devices=N_CORES)

    xT = nc.dram_tensor("xT", [ND, 128, R], dt.bfloat16, kind="ExternalInput").ap()
    w1d = nc.dram_tensor("w1d", [E, 128, NHL, ND, 128], dt.bfloat16, kind="ExternalInput").ap()
    w2d = nc.dram_tensor("w2d", [E, 128, ND, NHL, 128], dt.bfloat16, kind="ExternalInput").ap()
    b1d = nc.dram_tensor("b1d", [128, E, NHL], dt.float32, kind="ExternalInput").ap()
    outT = nc.dram_tensor("outT", [ND, 128, R], dt.bfloat16, kind="ExternalOutput").ap()

    with tile.TileContext(nc) as tc:
        with tc.tile_pool(name="const", bufs=1) as const, \
             tc.tile_pool(name="res", bufs=1) as res, \
             tc.tile_pool(name="xgp", bufs=4) as xgp, \
             tc.tile_pool(name="hp", bufs=2) as hp, \
             tc.tile_pool(name="op", bufs=6) as op, \
             tc.tile_pool(name="p1", bufs=3, space="PSUM") as p1, \
             tc.tile_pool(name="p2", bufs=4, space="PSUM") as p2:

            b1sb = const.tile([128, E, NHL], dt.float32)
            w1sb = res.tile([128, E, NHL, ND, 128], dt.bfloat16)   # 64KB/part
            w2sb = res.tile([128, E, ND, NHL, 128], dt.bfloat16)   # 64KB/part

            x_tiles = {}

            def load_x(k):
                _, s, L = sts[k]
                xg = xgp.tile([128, ND, LT], dt.bfloat16, tag="xg")
                nc.sync.dma_start(xg[:, :, :L], xT[:, :, s:s + L].transpose([1, 0, 2]))
                x_tiles[k] = xg

            # startup: first sub-tile's x + first expert's W1 chunks first so
            # the PE starts within ~3us; then the rest in consumption order.
            load_x(0)
            for hc in range(NHL):
                nc.sync.dma_start(w1sb[:, 0, hc], w1d[0, :, hc])
            nc.sync.dma_start(b1sb[:], b1d[:])
            load_x(1)
            nc.sync.dma_start(w2sb[:, 0], w2d[0])
            load_x(2)
            load_x(3)
            for e in range(1, E):
                nc.sync.dma_start(w1sb[:, e], w1d[e])
                nc.sync.dma_start(w2sb[:, e], w2d[e])

            h_tiles = {}

            def emit_l1(k):
                e, _, L = sts[k]
                xg = x_tiles[k]
                h = hp.tile([128, NHL, LT], dt.bfloat16, tag="h")
                for hc in range(NHL):
                    ps = p1.tile([128, LT], dt.float32, tag="ps1")
                    for dc in range(ND):
                        nc.tensor.matmul(ps[:, :L], lhsT=w1sb[:, e, hc, dc, :],
                                         rhs=xg[:, dc, :L],
                                         start=(dc == 0), stop=(dc == ND - 1))
                    nc.scalar.activation(h[:, hc, :L], ps[:, :L],
                                         bass.mybir.ActivationFunctionType.Gelu,
                                         bias=b1sb[:, e, hc:hc + 1])
                h_tiles[k] = h

            def emit_l2(k):
                e, s, L = sts[k]
                h = h_tiles.pop(k)
                for dc in range(ND):
                    ps = p2.tile([128, LT], dt.float32, tag="ps2")
                    for hc in range(NHL):
                        nc.tensor.matmul(ps[:, :L], lhsT=w2sb[:, e, dc, hc, :],
                                         rhs=h[:, hc, :L],
                                         start=(hc == 0), stop=(hc == NHL - 1))
                    osb = op.tile([128, LT], dt.bfloat16, tag="osb")
                    nc.vector.tensor_scalar_mul(osb[:, :L], ps[:, :L], 1.0)
                    nc.sync.dma_start(outT[dc, :, s:s + L], osb[:, :L])

            emit_l1(0)
            for k in range(n):
                if k + 4 < n:
                    load_x(k + 4)
                if k + 1 < n:
                    emit_l1(k + 1)
                emit_l2(k)

    nc.compile()
    return nc


def _route(x_flat, Wr):
    logits = x_flat @ Wr                                  # [T, E] fp32
    order = np.argsort(-logits, axis=1)
    top2 = order[:, :K]
    gap = (np.take_along_axis(logits, top2[:, 0:1], 1)
           - np.take_along_axis(logits, top2[:, 1:2], 1))[:, 0]
    w1v = 1.0 / (1.0 + np.exp(-gap))                      # softmax over top-2
    w2v = 1.0 - w1v
    idxs, wts = [], []
    for e in range(E):
        sel = (top2[:, 0] == e) | (top2[:, 1] == e)
        idx = np.nonzero(sel)[0]
        idxs.append(idx)
        wts.append(np.where(top2[idx, 0] == e, w1v[idx], w2v[idx]).astype(np.float32))
    combine = np.zeros((x_flat.shape[0], E), np.float32)
    np.put_along_axis(combine, top2[:, 0:1], w1v[:, None].astype(np.float32), 1)
    np.put_along_axis(combine, top2[:, 1:2], w2v[:, None].astype(np.float32), 1)
    return idxs, wts, combine


def kernel(x, Wr, W1, b1, W2, b2, _profile=None):
    global _compiled
    from concourse.bass_utils import run_bass_kernel_spmd

    x_flat = np.ascontiguousarray(np.asarray(x, np.float32)).reshape(T, DIM)
    idxs, wts, combine = _route(x_flat, np.asarray(Wr, np.float32))
    cnts = [len(i) for i in idxs]
    G = tuple(-(-c // 16) * 16 for c in cnts)
    R = sum(G)
    off = np.cumsum([0] + list(G))

    if G not in _compiled:
        _compiled[G] = _build(G)
    nc = _compiled[G]

    W1 = np.asarray(W1, np.float32)
    b1 = np.asarray(b1, np.float32)
    W2 = np.asarray(W2, np.float32)
    b2 = np.asarray(b2, np.float32)

    # gathered token stream, transposed: xT[dc, dp, row]
    Xg = np.zeros((R, DIM), np.float32)
    for e in range(E):
        Xg[off[e]:off[e] + cnts[e]] = x_flat[idxs[e]]
    xT = np.ascontiguousarray(Xg.reshape(R, ND, 128).transpose(1, 2, 0).astype(BF16))

    b1r = b1.reshape(E, N_CORES, NHL, 128)                # [e, core, hc, hp]
    in_maps = []
    for c in range(N_CORES):
        sl = slice(c * HS, (c + 1) * HS)
        w1c = W1[:, :, sl].astype(BF16).reshape(E, ND, 128, NHL, 128)
        w1c = np.ascontiguousarray(w1c.transpose(0, 2, 3, 1, 4))   # [e,dp,hc,dc,hp]
        w2c = W2[:, sl, :].astype(BF16).reshape(E, NHL, 128, ND, 128)
        w2c = np.ascontiguousarray(w2c.transpose(0, 2, 3, 1, 4))   # [e,hp,dc,hc,dp]
        b1c = np.ascontiguousarray(b1r[:, c].transpose(2, 0, 1))   # [hp,e,hc]
        in_maps.append({"xT": xT, "w1d": w1c, "w2d": w2c, "b1d": b1c})

    kwargs = {}
    if _profile:
        kwargs = dict(trace=True, tmpdir=_profile)
    res = run_bass_kernel_spmd(nc, in_maps, core_ids=list(range(N_CORES)), **kwargs)

    acc = np.zeros((ND, 128, R), np.float32)
    for c in range(N_CORES):
        acc += np.asarray(res.results[c]["outT"]).astype(np.float32)
    y = acc.transpose(2, 0, 1).reshape(R, DIM)

    full = combine @ b2                                    # [T, D] bias term
    for e in range(E):
        full[idxs[e]] += wts[e][:, None] * y[off[e]:off[e] + cnts[e]]
    full = full.reshape(4, 2048, DIM)
    if _profile:
        return full, res
    return full


# revision 6
# speedup vs baseline: 1.1298x; 1.0546x over previous
"""MoE layer (top-2 routing, 8 experts) on 8 Trainium2 NeuronCores.

Hidden-dim sharding for perfect load balance: every core holds an H/8
slice (512 wide) of ALL 8 experts' W1/W2 (16.8 MB bf16, SBUF-resident)
and processes ALL 16384 (token, expert) pairs for its slice — so the
per-core matmul column count is exactly 16384*64 regardless of routing
imbalance, and the SPMD program is bit-identical across cores (only
the weight-slice contents differ per core).

The host computes router logits (it must anyway, to decide the
dispatch), gathers tokens by expert into one x^T stream, and applies
the top-2 combine weights + cross-core reduction during the return
scatter-add.  The device runs only the FFN: per <=512-token sub-tile,
GEMM1 (4 h-chunk chains x 8 d-matmuls) -> exact-GELU(+b1) on the
scalar engine -> GEMM2 (8 d-chunk chains x 4 h-matmuls) -> bf16 cast
on the vector engine -> y^T DMA out.  l1(k+1) is emitted before l2(k)
(one-deep software pipeline) so the PE never waits on the gelu drain.
"""

import sys, os

for _p in ("/root/.axon_site", "/root/.axon_site/_ro/trn_rl_repo",
           "/root/.axon_site/_ro/pypackages", "/opt/trn_rl_repo"):
    if os.path.isdir(_p) and _p not in sys.path:
        sys.path.append(_p)

import numpy as np
import ml_dtypes

BF16 = ml_dtypes.bfloat16

T, DIM, E, K, H = 8192, 1024, 8, 2, 4096
N_CORES = 8
ND = DIM // 128             # 8 d-chunks
HS = H // N_CORES           # 512 hidden dims per core
NHL = HS // 128             # 4 h-chunks per core
LT = 512                    # sub-tile token length

_compiled = {}


def _build(G):
    from concourse import bass, bacc, tile, mybir

    dt = mybir.dt
    R = sum(G)

    # sub-tiles: (expert, global row start, length) — lengths evened out
    # (multiples of 16) so there are no tiny remainder chains at group ends
    sts = []
    off = 0
    for e in range(E):
        nst = -(-G[e] // LT)
        base = (G[e] // nst) // 16 * 16
        nplus = (G[e] - base * nst) // 16
        lens = [base + 16] * nplus + [base] * (nst - nplus)
        s = 0
        for L in lens:
            sts.append((e, off + s, L))
            s += L
        assert s == G[e]
        off += G[e]
    n = len(sts)

    nc = bacc.Bacc("TRN2", target_bir_lowering=False, debug=False,
                   num_devices=N_CORES)

    xT = nc.dram_tensor("xT", [ND, 128, R], dt.bfloat16, kind="ExternalInput").ap()
    w1d = nc.dram_tensor("w1d", [E, 128, NHL, ND, 128], dt.bfloat16, kind="ExternalInput").ap()
    w2d = nc.dram_tensor("w2d", [E, 128, ND, NHL, 128], dt.bfloat16, kind="ExternalInput").ap()
    b1d = nc.dram_tensor("b1d", [128, E, NHL], dt.float32, kind="ExternalInput").ap()
    outT = nc.dram_tensor("outT", [ND, 128, R], dt.bfloat16, kind="ExternalOutput").ap()

    with tile.TileContext(nc) as tc:
        with tc.tile_pool(name="const", bufs=1) as const, \
             tc.tile_pool(name="res", bufs=1) as res, \
             tc.tile_pool(name="xgp", bufs=4) as xgp, \
             tc.tile_pool(name="hp", bufs=2) as hp, \
             tc.tile_pool(name="op", bufs=6) as op, \
             tc.tile_pool(name="p1", bufs=3, space="PSUM") as p1, \
             tc.tile_pool(name="p2", bufs=5, space="PSUM") as p2:

            b1sb = const.tile([128, E, NHL], dt.float32)
            w1sb = res.tile([128, E, NHL, ND, 128], dt.bfloat16)   # 64KB/part
            w2sb = res.tile([128, E, ND, NHL, 128], dt.bfloat16)   # 64KB/part

            x_tiles = {}

            def load_x(k):
                _, s, L = sts[k]
                xg = xgp.tile([128, ND, LT], dt.bfloat16, tag="xg")
                nc.sync.dma_start(xg[:, :, :L], xT[:, :, s:s + L].transpose([1, 0, 2]))
                x_tiles[k] = xg

            # startup: first sub-tile's x + first expert's W1 chunks first so
            # the PE starts within ~3us.  Remaining experts' weights are
            # chunked and drip-fed inside the main loop (after each x
            # prefetch) so x DMAs never queue behind megabytes of weights
            # on the FIFO ring — that head-of-line block starved the PE
            # for ~28us and let the HAM clock-gate drop it to half rate.
            load_x(0)
            for hc in range(NHL):
                nc.sync.dma_start(w1sb[:, 0, hc], w1d[0, :, hc])
            nc.sync.dma_start(b1sb[:], b1d[:])
            load_x(1)
            nc.sync.dma_start(w2sb[:, 0], w2d[0])
            load_x(2)
            nc.sync.dma_start(w1sb[:, 1], w1d[1])
            load_x(3)
            wq = []                       # chunked weight DMAs, consumption order
            for e in range(1, E):
                if e > 1:
                    for hc in range(NHL):
                        wq.append((w1sb[:, e, hc], w1d[e, :, hc]))
                for dc in range(0, ND, 2):
                    wq.append((w2sb[:, e, dc:dc + 2], w2d[e, :, dc:dc + 2]))
            wq.reverse()                  # pop from the end

            h_tiles = {}

            def emit_l1(k):
                e, _, L = sts[k]
                xg = x_tiles[k]
                h = hp.tile([128, NHL, LT], dt.bfloat16, tag="h")
                for hc in range(NHL):
                    ps = p1.tile([128, LT], dt.float32, tag="ps1")
                    for dc in range(ND):
                        nc.tensor.matmul(ps[:, :L], lhsT=w1sb[:, e, hc, dc, :],
                                         rhs=xg[:, dc, :L],
                                         start=(dc == 0), stop=(dc == ND - 1))
                    nc.scalar.activation(h[:, hc, :L], ps[:, :L],
                                         bass.mybir.ActivationFunctionType.Gelu,
                                         bias=b1sb[:, e, hc:hc + 1])
                h_tiles[k] = h

            def emit_l2(k):
                e, s, L = sts[k]
                h = h_tiles.pop(k)
                for dc in range(ND):
                    ps = p2.tile([128, LT], dt.float32, tag="ps2")
                    for hc in range(NHL):
                        nc.tensor.matmul(ps[:, :L], lhsT=w2sb[:, e, dc, hc, :],
                                         rhs=h[:, hc, :L],
                                         start=(hc == 0), stop=(hc == NHL - 1))
                    osb = op.tile([128, LT], dt.bfloat16, tag="osb")
                    nc.vector.tensor_scalar_mul(osb[:, :L], ps[:, :L], 1.0)
                    nc.sync.dma_start(outT[dc, :, s:s + L], osb[:, :L])

            emit_l1(0)
            for k in range(n):
                if k + 4 < n:
                    load_x(k + 4)
                for _ in range(4):
                    if wq:
                        dst, src = wq.pop()
                        nc.sync.dma_start(dst, src)
                if k + 1 < n:
                    emit_l1(k + 1)
                emit_l2(k)

    nc.compile()
    return nc


def _route(x_flat, Wr):
    logits = x_flat @ Wr                                  # [T, E] fp32
    order = np.argsort(-logits, axis=1)
    top2 = order[:, :K]
    gap = (np.take_along_axis(logits, top2[:, 0:1], 1)
           - np.take_along_axis(logits, top2[:, 1:2], 1))[:, 0]
    w1v = 1.0 / (1.0 + np.exp(-gap))                      # softmax over top-2
    w2v = 1.0 - w1v
    idxs, wts = [], []
    for e in range(E):
        sel = (top2[:, 0] == e) | (top2[:, 1] == e)
        idx = np.nonzero(sel)[0]
        idxs.append(idx)
        wts.append(np.where(top2[idx, 0] == e, w1v[idx], w2v[idx]).astype(np.float32))
    combine = np.zeros((x_flat.shape[0], E), np.float32)
    np.put_along_axis(combine, top2[:, 0:1], w1v[:, None].astype(np.float32), 1)
    np.put_along_axis(combine, top2[:, 1:2], w2v[:, None].astype(np.float32), 1)
    return idxs, wts, combine


def kernel(x, Wr, W1, b1, W2, b2, _profile=None):
    global _compiled
    from concourse.bass_utils import run_bass_kernel_spmd

    x_flat = np.ascontiguousarray(np.asarray(x, np.float32)).reshape(T, DIM)
    idxs, wts, combine = _route(x_flat, np.asarray(Wr, np.float32))
    cnts = [len(i) for i in idxs]
    G = tuple(-(-c // 16) * 16 for c in cnts)
    R = sum(G)
    off = np.cumsum([0] + list(G))

    if G not in _compiled:
        _compiled[G] = _build(G)
    nc = _compiled[G]

    W1 = np.asarray(W1, np.float32)
    b1 = np.asarray(b1, np.float32)
    W2 = np.asarray(W2, np.float32)
    b2 = np.asarray(b2, np.float32)

    # gathered token stream, transposed: xT[dc, dp, row]
    Xg = np.zeros((R, DIM), np.float32)
    for e in range(E):
        Xg[off[e]:off[e] + cnts[e]] = x_flat[idxs[e]]
    xT = np.ascontiguousarray(Xg.reshape(R, ND, 128).transpose(1, 2, 0).astype(BF16))

    b1r = b1.reshape(E, N_CORES, NHL, 128)                # [e, core, hc, hp]
    in_maps = []
    for c in range(N_CORES):
        sl = slice(c * HS, (c + 1) * HS)
        w1c = W1[:, :, sl].astype(BF16).reshape(E, ND, 128, NHL, 128)
        w1c = np.ascontiguousarray(w1c.transpose(0, 2, 3, 1, 4))   # [e,dp,hc,dc,hp]
        w2c = W2[:, sl, :].astype(BF16).reshape(E, NHL, 128, ND, 128)
        w2c = np.ascontiguousarray(w2c.transpose(0, 2, 3, 1, 4))   # [e,hp,dc,hc,dp]
        b1c = np.ascontiguousarray(b1r[:, c].transpose(2, 0, 1))   # [hp,e,hc]
        in_maps.append({"xT": xT, "w1d": w1c, "w2d": w2c, "b1d": b1c})

    kwargs = {}
    if _profile:
        kwargs = dict(trace=True, tmpdir=_profile)
    res = run_bass_kernel_spmd(nc, in_maps, core_ids=list(range(N_CORES)), **kwargs)

    acc = np.zeros((ND, 128, R), np.float32)
    for c in range(N_CORES):
        acc += np.asarray(res.results[c]["outT"]).astype(np.float32)
    y = acc.transpose(2, 0, 1).reshape(R, DIM)

    full = combine @ b2                                    # [T, D] bias term
    for e in range(E):
        full[idxs[e]] += wts[e][:, None] * y[off[e]:off[e] + cnts[e]]
    full = full.reshape(4, 2048, DIM)
    if _profile:
        return full, res
    return full
